# revision 8
# baseline (speedup 1.0000x reference)
import sys
sys.path.insert(0, "/opt/trn_rl_repo")
import numpy as np
import concourse.bass as bass
import concourse.tile as tile
from concourse import bacc, mybir
from concourse import bass_utils
from concourse.masks import make_identity

f32, f32r, bf16 = mybir.dt.float32, mybir.dt.float32r, mybir.dt.bfloat16
AF = mybir.ActivationFunctionType
AX = mybir.AxisListType

# problem dims (hardcoded)
P, B, S, D = 2, 2, 2048, 1024
H, DH, R = 16, 64, 16
DS, DE = 64, 64
NFQK, NFV, NRQK, NRV = 32, 32, 16, 16
NC_ = 8
T = (B * S) // NC_          # 512 rows per core
NEG = -1.0e30


def _bc(ap2d, reps):
    """insert a broadcast (step-0) middle free dim into a [p, n] AP"""
    return bass.AP(tensor=ap2d.tensor, offset=ap2d.offset,
                   ap=[ap2d.ap[0], [0, reps], ap2d.ap[-1]])


def _bc_last(ap2d, reps):
    """append a broadcast (step-0) innermost free dim to a [p, n] AP"""
    return bass.AP(tensor=ap2d.tensor, offset=ap2d.offset,
                   ap=[ap2d.ap[0], ap2d.ap[-1], [0, reps]])


# ---------------------------------------------------------------- launch A
def build_A():
    nc = bacc.Bacc("TRN2", target_bir_lowering=False, debug=False,
                   enable_asserts=False, num_devices=NC_)
    dt_ = nc.dram_tensor
    xT = dt_("xT", [D, T], f32r, kind="ExternalInput").ap()
    Fa = dt_("Fa", [D, 1024], f32r, kind="ExternalInput").ap()
    Rqk = dt_("Rqk", [256, D], f32r, kind="ExternalInput").ap()
    Rv = dt_("Rv", [256, D], f32r, kind="ExternalInput").ap()
    wf = dt_("wf", [3, P, T, 32], f32, kind="ExternalInput").ap()
    wfT = dt_("wfT", [3, 32, P * T], f32, kind="ExternalInput").ap()
    emb = dt_("emb", [2, 32, DE], f32, kind="ExternalInput").ap()
    Wp = dt_("Wp", [2, 16, DS], f32, kind="ExternalInput").ap()
    Wr = dt_("Wr", [2, 128, 16], f32, kind="ExternalInput").ap()
    bp = dt_("bp", [2, DS, 1], f32, kind="ExternalInput").ap()
    QTo = dt_("QTo", [D, T], f32, kind="ExternalOutput").ap()
    KTo = dt_("KTo", [D, T], f32, kind="ExternalOutput").ap()
    Vo = dt_("Vo", [T, D], f32, kind="ExternalOutput").ap()
    WSo = dt_("WSo", [1, 48], f32, kind="ExternalOutput").ap()

    with tile.TileContext(nc) as tc:
        with tc.tile_pool(name="cst", bufs=1) as cst, \
             tc.tile_pool(name="big", bufs=2) as big, \
             tc.tile_pool(name="ps", bufs=2, space="PSUM") as ps, \
             tc.tile_pool(name="ps1", bufs=1, space="PSUM") as ps1:

            xs = cst.tile([128, 8, T], f32r, name="xs")
            nc.sync.dma_start(out=xs, in_=xT.rearrange("(c p) t -> p c t", p=128))
            fa = cst.tile([128, 8, 1024], f32r, name="fa")
            nc.sync.dma_start(out=fa, in_=Fa.rearrange("(c p) t -> p c t", p=128))
            rq = cst.tile([128, 2, D], f32r, name="rq")
            nc.sync.dma_start(out=rq, in_=Rqk.rearrange("(g p) d -> p g d", p=128))
            rv = cst.tile([128, 2, D], f32r, name="rv")
            nc.sync.dma_start(out=rv, in_=Rv.rearrange("(g p) d -> p g d", p=128))
            wfs = cst.tile([128, 3, P, 4, 32], f32, name="wfs")
            nc.sync.dma_start(out=wfs, in_=wf.rearrange("w p (t q) n -> q w p t n", q=128))
            wfts = cst.tile([32, 3, P * T], f32, name="wfts")
            nc.sync.dma_start(out=wfts, in_=wfT.rearrange("w n c -> n w c"))
            embs = cst.tile([32, 2, DE], f32, name="embs")
            nc.sync.dma_start(out=embs, in_=emb.rearrange("w n c -> n w c"))
            wps = cst.tile([16, 2, DS], f32, name="wps")
            nc.sync.dma_start(out=wps, in_=Wp.rearrange("w n c -> n w c"))
            wrs = cst.tile([128, 2, 16], f32, name="wrs")
            nc.sync.dma_start(out=wrs, in_=Wr.rearrange("w n c -> n w c"))
            bps = cst.tile([DS, 2, 1], f32, name="bps")
            nc.sync.dma_start(out=bps, in_=bp.rearrange("w n c -> n w c"))
            id32 = cst.tile([128, 128], f32, name="id32")
            make_identity(nc, id32)
            id32r = cst.tile([128, 128], f32r, name="id32r")
            nc.vector.tensor_copy(id32r, id32)
            ones128 = cst.tile([128, 1], f32, name="ones128")
            nc.vector.memset(ones128, 1.0)

            # features: all_h tiles [128,1024] per row-tile (kept alive)
            ahs = []
            for t in range(4):
                pah = ps.tile([128, 1024], f32, tag="pah", name=f"pah{t}")
                for c in range(8):
                    for hh in range(2):
                        nc.tensor.matmul(
                            pah[:, hh * 512:(hh + 1) * 512],
                            xs[:, c, t * 128:(t + 1) * 128],
                            fa[:, c, hh * 512:(hh + 1) * 512],
                            start=(c == 0), stop=(c == 7))
                ah = cst.tile([128, 1024], f32, name=f"ah{t}")
                nc.scalar.copy(ah, pah)
                ahs.append(ah)

            wsp_all = ps1.tile([1, 48], f32, tag="wsp", name="wsp_all")
            for w in range(3):  # Q, K, V routers
                half = 0 if w < 2 else 1           # qk / v feature half
                rt = 0 if w < 2 else 1             # router type qk / v
                # mixing -> h tiles
                hts = {}
                for t in range(4):
                    for p in range(P):
                        tmp = big.tile([128, 512], f32, tag="tmp", name=f"tmp{w}{t}{p}")
                        src = ahs[t][:, half * 512:(half + 1) * 512]
                        src3 = src.rearrange("q (n r) -> q r n", r=16)
                        wtile = wfs[:, w, p, t, :]
                        nc.vector.tensor_mul(
                            tmp.rearrange("q (r n) -> q r n", n=32), src3, _bc(wtile, 16))
                        hh = big.tile([128, 16], f32, tag="h", bufs=8,
                                      name=f"h{w}{t}{p}")
                        nc.vector.reduce_sum(
                            out=hh, in_=tmp.rearrange("q (r n) -> q r n", n=32), axis=AX.X)
                        hts[(t, p)] = hh
                # transpose h -> hT [16, P*T]
                hT = big.tile([16, P * T], f32, tag="hT", name=f"hT{w}")
                for t in range(4):
                    for p in range(P):
                        pt = ps.tile([16, 128], f32, tag="lg", name=f"pt{w}{t}{p}")
                        nc.tensor.transpose(pt, hts[(t, p)], id32)
                        nc.vector.tensor_copy(
                            hT[:, p * T + t * 128: p * T + (t + 1) * 128], pt)
                # projT / ctxT -> inT
                pj = ps.tile([64, P * T], f32, tag="pah", name=f"pj{w}")
                cx = ps.tile([64, P * T], f32, tag="pah", name=f"cx{w}")
                for sl in range(2):
                    s0 = sl * 512
                    nc.tensor.matmul(pj[:, s0:s0 + 512], wps[:, rt, :],
                                     hT[:, s0:s0 + 512], start=True, stop=True)
                    nc.tensor.matmul(cx[:, s0:s0 + 512], embs[:, rt, :],
                                     wfts[:, w, s0:s0 + 512], start=True, stop=True)
                inT = big.tile([128, P * T], f32, tag="inT", name=f"inT{w}")
                nc.scalar.activation(inT[0:64, :], pj, AF.Identity,
                                     bias=bps[:, rt, :], scale=1.0)
                nc.scalar.copy(inT[64:128, :], cx)
                # router softmax (no max subtraction; |logits| < 16)
                wsp = wsp_all[:, w * 16:(w + 1) * 16]
                wtiles = {}
                for u in range(8):
                    lg = ps.tile([128, 16], f32, tag="lg", name=f"lg{w}{u}")
                    nc.tensor.matmul(lg, inT[:, u * 128:(u + 1) * 128],
                                     wrs[:, rt, :], start=True, stop=True)
                    wun = big.tile([128, 16], f32, tag="wun", name=f"wun{w}{u}")
                    zr = big.tile([128, 1], f32, tag="zr", name=f"zr{w}{u}")
                    nc.scalar.activation(wun, lg, AF.Exp, bias=0.0, scale=1.0,
                                         accum_out=zr)
                    rz = big.tile([128, 1], f32, tag="rz", name=f"rz{w}{u}")
                    nc.vector.reciprocal(rz, zr)
                    wt = big.tile([128, 16], f32, tag="wt", bufs=8, name=f"wt{w}{u}")
                    nc.vector.tensor_scalar_mul(wt, wun, rz)
                    nc.tensor.matmul(wsp, ones128, wt, start=(u == 0), stop=(u == 7))
                    wtiles[u] = wt
                # t-product + transpose -> ttT [2][128, P*T]
                ttT = [big.tile([128, P * T], f32r, tag=f"ttT{g}", name=f"ttT{w}{g}")
                       for g in range(2)]
                for t in range(4):
                    for p in range(P):
                        u = p * 4 + t
                        tt = big.tile([128, 256], f32r, tag="tt", name=f"tt{w}{t}{p}")
                        wt = wtiles[u]
                        hh = hts[(t, p)]
                        nc.vector.tensor_mul(
                            tt.rearrange("q (n r) -> q n r", r=16),
                            _bc_last(wt, 16), _bc(hh, 16))
                        for g in range(2):
                            ptt = ps.tile([128, 128], f32r, tag="lg",
                                          name=f"ptt{w}{t}{p}{g}")
                            nc.tensor.transpose(
                                ptt, tt[:, g * 128:(g + 1) * 128], id32r)
                            nc.vector.tensor_copy(
                                ttT[g][:, p * T + t * 128: p * T + (t + 1) * 128], ptt)
                # restore
                rr = rq if w < 2 else rv
                if w < 2:  # Q^T / K^T  [d, row] (sum over p)
                    out = QTo if w == 0 else KTo
                    for c in range(8):
                        qp = ps.tile([128, T], f32, tag="pah", name=f"qp{w}{c}")
                        k = 0
                        for g in range(2):
                            for p in range(P):
                                nc.tensor.matmul(
                                    qp, rr[:, g, c * 128:(c + 1) * 128],
                                    ttT[g][:, p * T:(p + 1) * T],
                                    start=(k == 0), stop=(k == 3))
                                k += 1
                        qs = big.tile([128, T], f32, tag="qs", name=f"qs{w}{c}")
                        nc.scalar.copy(qs, qp)
                        nc.sync.dma_start(out=out[c * 128:(c + 1) * 128, :], in_=qs)
                else:  # V natural [row, d] (sum over p)
                    for t in range(4):
                        vp = ps.tile([128, D], f32, tag="pah", name=f"vp{t}")
                        for sl in range(2):
                            s0 = sl * 512
                            k = 0
                            for g in range(2):
                                for p in range(P):
                                    nc.tensor.matmul(
                                        vp[:, s0:s0 + 512],
                                        ttT[g][:, p * T + t * 128: p * T + (t + 1) * 128],
                                        rr[:, g, s0:s0 + 512],
                                        start=(k == 0), stop=(k == 3))
                                    k += 1
                        vs = big.tile([128, D], f32, tag="vs", name=f"vs{t}")
                        nc.scalar.copy(vs, vp)
                        nc.sync.dma_start(out=Vo[t * 128:(t + 1) * 128, :], in_=vs)
            # aux partial sums
            wsb = cst.tile([1, 48], f32, name="wsb")
            nc.vector.tensor_copy(wsb, wsp_all)
            nc.sync.dma_start(out=WSo, in_=wsb)
    nc.compile()
    return nc


# ---------------------------------------------------------------- launch B
def build_B():
    nc = bacc.Bacc("TRN2", target_bir_lowering=False, debug=False,
                   enable_asserts=False, num_devices=NC_)
    dt_ = nc.dram_tensor
    QTa = dt_("QTa", [4, 65, S], f32r, kind="ExternalInput").ap()
    KTa = dt_("KTa", [4, 65, S], f32r, kind="ExternalInput").ap()
    Va = dt_("Va", [4, S, 65], f32r, kind="ExternalInput").ap()
    WOs = dt_("WOs", [2, 128, D], f32r, kind="ExternalInput").ap()
    TRI = dt_("TRI", [128, 128], f32, kind="ExternalInput").ap()    # -1e30 strict upper
    TRI01 = dt_("TRI01", [128, 128], f32, kind="ExternalInput").ap()  # -1e30 strict lower (k>q)
    OUT = dt_("OUT", [S, D], f32, kind="ExternalOutput").ap()

    NQT = S // 128  # 16
    with tile.TileContext(nc) as tc:
        with tc.tile_pool(name="cst", bufs=1) as cst, \
             tc.tile_pool(name="pr", bufs=2) as pr, \
             tc.tile_pool(name="sm", bufs=4) as sm, \
             tc.tile_pool(name="ps", bufs=2, space="PSUM") as ps, \
             tc.tile_pool(name="psu", bufs=1, space="PSUM") as psu:

            tri = cst.tile([128, 128], f32, name="tri")
            nc.sync.dma_start(out=tri, in_=TRI)
            tri01 = cst.tile([128, 128], f32, name="tri01")
            nc.sync.dma_start(out=tri01, in_=TRI01)
            id32 = cst.tile([128, 128], f32, name="id32")
            make_identity(nc, id32)
            ones64f = cst.tile([1, 64], f32, name="ones64f")
            nc.vector.memset(ones64f, 1.0)
            ones64 = cst.tile([1, 64], f32r, name="ones64")
            nc.vector.tensor_copy(ones64, ones64f)
            wos = [cst.tile([128, D], f32r, name=f"wos{ch}") for ch in range(2)]
            for ch in range(2):
                nc.sync.dma_start(out=wos[ch], in_=WOs[ch])
            aot = [cst.tile([128, S], f32r, name=f"aot{half}") for half in range(2)]

            for pair in range(4):
                qta = pr.tile([65, S], f32r, tag="qta", name=f"qta{pair}")
                nc.sync.dma_start(out=qta, in_=QTa[pair])
                kta = pr.tile([65, S], f32r, tag="kta", name=f"kta{pair}")
                nc.sync.dma_start(out=kta, in_=KTa[pair])
                va = pr.tile([128, 16, 65], f32r, tag="va", name=f"va{pair}")
                nc.sync.dma_start(out=va,
                                  in_=Va[pair].rearrange("(j p) c -> p j c", p=128))

                # ---- pass 1: true row-max per q-tile (scores in [q,k] layout)
                for i in range(NQT):
                    nk = 128 * (i + 1)
                    m01 = sm.tile([128, 2], f32, tag="m01", name=f"m01{pair}{i}")
                    nhalves = 1 if i < 8 else 2
                    if nhalves == 1:
                        nc.vector.memset(m01[:, 1:2], -3.0e38)
                    for hh in range(nhalves):
                        k0 = 1024 * hh
                        kw = min(1024, nk - k0)
                        s1 = ps.tile([128, 1024], f32, tag="s1",
                                     name=f"s1{pair}{i}{hh}")
                        for sl in range((kw + 511) // 512):
                            w_ = min(512, kw - sl * 512)
                            nc.tensor.matmul(
                                s1[:, sl * 512: sl * 512 + w_],
                                qta[0:64, i * 128:(i + 1) * 128],
                                kta[0:64, k0 + sl * 512: k0 + sl * 512 + w_],
                                start=True, stop=True)
                        if i // 8 == hh:  # diag block inside this half
                            off = 128 * i - k0
                            nc.vector.tensor_add(
                                s1[:, off:off + 128], s1[:, off:off + 128], tri)
                        nc.vector.reduce_max(out=m01[:, hh:hh + 1],
                                             in_=s1[:, 0:kw], axis=AX.X)
                    m = sm.tile([128, 1], f32, tag="m", name=f"m{pair}{i}")
                    nc.vector.reduce_max(out=m, in_=m01, axis=AX.X)
                    mt = ps.tile([1, 128], f32, tag="s1", name=f"mt{pair}{i}")
                    nc.tensor.transpose(mt, m, id32)
                    nc.vector.tensor_scalar_mul(
                        qta[64:65, i * 128:(i + 1) * 128], mt, -1.0)

                # ---- pass 2 + AV (scores transposed, max folded via aug row)
                U = psu.tile([65, S], f32, tag="U", name=f"U{pair}")
                for j in range(NQT):
                    q0 = 128 * j
                    first = True
                    qq = q0
                    while qq < S:
                        w_ = min(512, S - qq)
                        st = ps.tile([128, 512], f32, tag="s1",
                                     name=f"st{pair}{j}{qq}")
                        nc.tensor.matmul(st[:, :w_],
                                         kta[:, q0:q0 + 128],
                                         qta[:, qq:qq + w_],
                                         start=True, stop=True)
                        if first:
                            nc.vector.tensor_add(st[:, 0:128], st[:, 0:128], tri01)
                        et = sm.tile([128, 512], f32r, tag="et",
                                     name=f"et{pair}{j}{qq}")
                        nc.scalar.activation(et[:, :w_], st[:, :w_], AF.Exp,
                                             bias=0.0, scale=0.125)
                        nc.tensor.matmul(U[:, qq:qq + w_],
                                         va[:, j, :], et[:, :w_],
                                         start=(j == 0), stop=(j == NQT - 1))
                        first = False
                        qq += w_

                # ---- normalize -> aot
                half = pair // 2
                r0 = 64 * (pair % 2)
                for sl in range(4):
                    s0 = sl * 512
                    rz = sm.tile([1, 512], f32r, tag="rz", name=f"rz{pair}{sl}")
                    with nc.allow_low_precision(reason="f32r is f32 bits"):
                        nc.vector.reciprocal(rz, U[64:65, s0:s0 + 512])
                    bc = ps.tile([64, 512], f32, tag="s1", name=f"bc{pair}{sl}")
                    nc.tensor.matmul(bc, ones64, rz, start=True, stop=True)
                    bcs = sm.tile([64, 512], f32, tag="bcs", name=f"bcs{pair}{sl}")
                    nc.vector.tensor_copy(bcs, bc)
                    nc.vector.tensor_mul(
                        aot[half][r0:r0 + 64, s0:s0 + 512],
                        U[0:64, s0:s0 + 512], bcs)

            # ---- W_O partial
            for i in range(NQT):
                wo = ps.tile([128, D], f32, tag="s1", name=f"wo{i}")
                for sl in range(2):
                    s0 = sl * 512
                    for ch in range(2):
                        nc.tensor.matmul(wo[:, s0:s0 + 512],
                                         aot[ch][:, i * 128:(i + 1) * 128],
                                         wos[ch][:, s0:s0 + 512],
                                         start=(ch == 0), stop=(ch == 1))
                wsb = sm.tile([128, D], f32, tag="wsb", name=f"wsb{i}")
                nc.scalar.copy(wsb, wo)
                nc.sync.dma_start(out=OUT[i * 128:(i + 1) * 128, :], in_=wsb)
    nc.compile()
    return nc


_NC_A, _NC_B = None, None


def kernel(x, fqk_wQ, fqk_wK, fv_w, f_neurons, r_neurons, fqk_emb, fv_emb,
           Wp_qk, bp_qk, Wp_v, bp_v, Wr_qk, Wr_v, W_O):
    global _NC_A, _NC_B
    if _NC_A is None:
        _NC_A = build_A()
        _NC_B = build_B()

    f32n = np.float32
    x = np.asarray(x, f32n)
    xf = x.reshape(B * S, D)
    Fa = np.ascontiguousarray(
        np.asarray(f_neurons, f32n).transpose(1, 0, 2).reshape(D, 1024))
    rn = np.asarray(r_neurons, f32n)
    Rqk = np.ascontiguousarray(rn[:NRQK].reshape(256, D))
    Rv = np.ascontiguousarray(rn[NRQK:].reshape(256, D))
    wf_all = np.stack([np.asarray(fqk_wQ, f32n), np.asarray(fqk_wK, f32n),
                       np.asarray(fv_w, f32n)])          # [3,P,B,S,32]
    wf_flat = wf_all.reshape(3, P, B * S, 32)
    emb = np.stack([np.asarray(fqk_emb, f32n), np.asarray(fv_emb, f32n)])
    Wp = np.stack([np.asarray(Wp_qk, f32n), np.asarray(Wp_v, f32n)])
    Wr = np.stack([np.asarray(Wr_qk, f32n), np.asarray(Wr_v, f32n)])
    cb = np.stack([np.asarray(bp_qk, f32n).reshape(DS, 1),
                   np.asarray(bp_v, f32n).reshape(DS, 1)])

    in_maps_a = []
    for c in range(NC_):
        rows = slice(c * T, (c + 1) * T)
        wfc = np.ascontiguousarray(wf_flat[:, :, rows, :])           # [3,P,T,32]
        wfTc = np.ascontiguousarray(
            wfc.transpose(0, 3, 1, 2).reshape(3, 32, P * T))
        in_maps_a.append(dict(
            xT=np.ascontiguousarray(xf[rows].T), Fa=Fa, Rqk=Rqk, Rv=Rv,
            wf=wfc, wfT=wfTc, emb=emb, Wp=Wp, Wr=Wr, bp=cb))
    resA = bass_utils.run_bass_kernel_spmd(_NC_A, in_maps_a,
                                           core_ids=list(range(NC_)))

    QT = np.empty((B, D, S), f32n)
    KT = np.empty((B, D, S), f32n)
    V = np.empty((B, S, D), f32n)
    ws = np.zeros((3, 16), f32n)
    for c in range(NC_):
        b, cc = c // 4, c % 4
        QT[b][:, cc * T:(cc + 1) * T] = resA.results[c]["QTo"]
        KT[b][:, cc * T:(cc + 1) * T] = resA.results[c]["KTo"]
        V[b][cc * T:(cc + 1) * T, :] = resA.results[c]["Vo"]
        ws += resA.results[c]["WSo"].reshape(3, 16)
    mean_p = ws / f32n(P * B * S)
    aux = f32n(16.0) * (mean_p * mean_p).sum(axis=1)
    restore_aux_loss = np.float32(aux.sum())

    # q_norm scale (reference: 1 unless ||Q|| <= 1e-6)
    qn = np.sqrt((QT ** 2).sum(axis=1))                      # [B, S]
    scale = np.where(qn > 1e-6, f32n(1.0), qn * f32n(1e-6))

    TRIm = np.where(np.arange(128)[:, None] < np.arange(128)[None, :],
                    f32n(NEG), f32n(0.0)).astype(f32n)
    TRI01 = np.where(np.arange(128)[:, None] > np.arange(128)[None, :],
                     f32n(NEG), f32n(0.0)).astype(f32n)
    WOn = np.asarray(W_O, f32n)

    in_maps_b = []
    for c in range(NC_):
        b, hb = c // 4, c % 4
        qa = np.zeros((4, 65, S), f32n)
        ka = np.zeros((4, 65, S), f32n)
        va = np.zeros((4, S, 65), f32n)
        for t in range(4):
            h = 4 * hb + t
            qa[t, 0:64] = QT[b][h * DH:(h + 1) * DH]
            ka[t, 0:64] = KT[b][h * DH:(h + 1) * DH]
            ka[t, 64] = 1.0
            va[t, :, 0:64] = V[b][:, h * DH:(h + 1) * DH]
            va[t, :, 64] = 1.0
        wos = np.ascontiguousarray(
            WOn[hb * 256:(hb + 1) * 256].reshape(2, 128, D))
        in_maps_b.append(dict(QTa=qa, KTa=ka, Va=va, WOs=wos,
                              TRI=TRIm, TRI01=TRI01))
    resB = bass_utils.run_bass_kernel_spmd(_NC_B, in_maps_b,
                                           core_ids=list(range(NC_)))

    out = np.zeros((B, S, D), f32n)
    for c in range(NC_):
        out[c // 4] += resB.results[c]["OUT"]
    if not np.all(scale == 1.0):
        out *= scale[:, :, None]
    return out, restore_aux_loss


# revision 10
# speedup vs baseline: 1.1536x; 1.1536x over previous
import sys
sys.path.insert(0, "/opt/trn_rl_repo")
import numpy as np
import concourse.bass as bass
import concourse.tile as tile
from concourse import bacc, mybir
from concourse import bass_utils
from concourse.masks import make_identity

f32, f32r, bf16 = mybir.dt.float32, mybir.dt.float32r, mybir.dt.bfloat16
AF = mybir.ActivationFunctionType
AX = mybir.AxisListType

# problem dims (hardcoded)
P, B, S, D = 2, 2, 2048, 1024
H, DH, R = 16, 64, 16
DS, DE = 64, 64
NFQK, NFV, NRQK, NRV = 32, 32, 16, 16
NC_ = 8
T = (B * S) // NC_          # 512 rows per core
NEG = -1.0e30


def _bc(ap2d, reps):
    """insert a broadcast (step-0) middle free dim into a [p, n] AP"""
    return bass.AP(tensor=ap2d.tensor, offset=ap2d.offset,
                   ap=[ap2d.ap[0], [0, reps], ap2d.ap[-1]])


def _bc_last(ap2d, reps):
    """append a broadcast (step-0) innermost free dim to a [p, n] AP"""
    return bass.AP(tensor=ap2d.tensor, offset=ap2d.offset,
                   ap=[ap2d.ap[0], ap2d.ap[-1], [0, reps]])


# ---------------------------------------------------------------- launch A
def build_A():
    nc = bacc.Bacc("TRN2", target_bir_lowering=False, debug=False,
                   enable_asserts=False, num_devices=NC_)
    dt_ = nc.dram_tensor
    xT = dt_("xT", [D, T], f32r, kind="ExternalInput").ap()
    Fa = dt_("Fa", [D, 1024], f32r, kind="ExternalInput").ap()
    Rqk = dt_("Rqk", [256, D], f32r, kind="ExternalInput").ap()
    Rv = dt_("Rv", [256, D], f32r, kind="ExternalInput").ap()
    wf = dt_("wf", [3, P, T, 32], f32, kind="ExternalInput").ap()
    wfT = dt_("wfT", [3, 32, P * T], f32, kind="ExternalInput").ap()
    emb = dt_("emb", [2, 32, DE], f32, kind="ExternalInput").ap()
    Wp = dt_("Wp", [2, 16, DS], f32, kind="ExternalInput").ap()
    Wr = dt_("Wr", [2, 128, 16], f32, kind="ExternalInput").ap()
    bp = dt_("bp", [2, DS, 1], f32, kind="ExternalInput").ap()
    QTo = dt_("QTo", [D, T], f32, kind="ExternalOutput").ap()
    KTo = dt_("KTo", [D, T], f32, kind="ExternalOutput").ap()
    Vo = dt_("Vo", [T, D], f32, kind="ExternalOutput").ap()
    WSo = dt_("WSo", [1, 48], f32, kind="ExternalOutput").ap()

    with tile.TileContext(nc) as tc:
        with tc.tile_pool(name="cst", bufs=1) as cst, \
             tc.tile_pool(name="big", bufs=2) as big, \
             tc.tile_pool(name="ps", bufs=2, space="PSUM") as ps, \
             tc.tile_pool(name="ps1", bufs=1, space="PSUM") as ps1:

            xs = cst.tile([128, 8, T], f32r, name="xs")
            nc.sync.dma_start(out=xs, in_=xT.rearrange("(c p) t -> p c t", p=128))
            fa = cst.tile([128, 8, 1024], f32r, name="fa")
            nc.sync.dma_start(out=fa, in_=Fa.rearrange("(c p) t -> p c t", p=128))
            rq = cst.tile([128, 2, D], f32r, name="rq")
            nc.sync.dma_start(out=rq, in_=Rqk.rearrange("(g p) d -> p g d", p=128))
            rv = cst.tile([128, 2, D], f32r, name="rv")
            nc.sync.dma_start(out=rv, in_=Rv.rearrange("(g p) d -> p g d", p=128))
            wfs = cst.tile([128, 3, P, 4, 32], f32, name="wfs")
            nc.sync.dma_start(out=wfs, in_=wf.rearrange("w p (t q) n -> q w p t n", q=128))
            wfts = cst.tile([32, 3, P * T], f32, name="wfts")
            nc.sync.dma_start(out=wfts, in_=wfT.rearrange("w n c -> n w c"))
            embs = cst.tile([32, 2, DE], f32, name="embs")
            nc.sync.dma_start(out=embs, in_=emb.rearrange("w n c -> n w c"))
            wps = cst.tile([16, 2, DS], f32, name="wps")
            nc.sync.dma_start(out=wps, in_=Wp.rearrange("w n c -> n w c"))
            wrs = cst.tile([128, 2, 16], f32, name="wrs")
            nc.sync.dma_start(out=wrs, in_=Wr.rearrange("w n c -> n w c"))
            bps = cst.tile([DS, 2, 1], f32, name="bps")
            nc.sync.dma_start(out=bps, in_=bp.rearrange("w n c -> n w c"))
            id32 = cst.tile([128, 128], f32, name="id32")
            make_identity(nc, id32)
            id32r = cst.tile([128, 128], f32r, name="id32r")
            nc.vector.tensor_copy(id32r, id32)
            ones128 = cst.tile([128, 1], f32, name="ones128")
            nc.vector.memset(ones128, 1.0)

            # features: all_h tiles [128,1024] per row-tile (kept alive)
            ahs = []
            for t in range(4):
                pah = ps.tile([128, 1024], f32, tag="pah", name=f"pah{t}")
                for c in range(8):
                    for hh in range(2):
                        nc.tensor.matmul(
                            pah[:, hh * 512:(hh + 1) * 512],
                            xs[:, c, t * 128:(t + 1) * 128],
                            fa[:, c, hh * 512:(hh + 1) * 512],
                            start=(c == 0), stop=(c == 7))
                ah = cst.tile([128, 1024], f32, name=f"ah{t}")
                nc.scalar.copy(ah, pah)
                ahs.append(ah)

            wsp_all = ps1.tile([1, 48], f32, tag="wsp", name="wsp_all")
            for w in range(3):  # Q, K, V routers
                half = 0 if w < 2 else 1           # qk / v feature half
                rt = 0 if w < 2 else 1             # router type qk / v
                # mixing -> h tiles
                hts = {}
                for t in range(4):
                    for p in range(P):
                        tmp = big.tile([128, 512], f32, tag="tmp", name=f"tmp{w}{t}{p}")
                        src = ahs[t][:, half * 512:(half + 1) * 512]
                        src3 = src.rearrange("q (n r) -> q r n", r=16)
                        wtile = wfs[:, w, p, t, :]
                        nc.vector.tensor_mul(
                            tmp.rearrange("q (r n) -> q r n", n=32), src3, _bc(wtile, 16))
                        hh = big.tile([128, 16], f32, tag="h", bufs=8,
                                      name=f"h{w}{t}{p}")
                        nc.vector.reduce_sum(
                            out=hh, in_=tmp.rearrange("q (r n) -> q r n", n=32), axis=AX.X)
                        hts[(t, p)] = hh
                # transpose h -> hT [16, P*T]
                hT = big.tile([16, P * T], f32, tag="hT", name=f"hT{w}")
                for t in range(4):
                    for p in range(P):
                        pt = ps.tile([16, 128], f32, tag="lg", name=f"pt{w}{t}{p}")
                        nc.tensor.transpose(pt, hts[(t, p)], id32)
                        nc.vector.tensor_copy(
                            hT[:, p * T + t * 128: p * T + (t + 1) * 128], pt)
                # projT / ctxT -> inT
                pj = ps.tile([64, P * T], f32, tag="pah", name=f"pj{w}")
                cx = ps.tile([64, P * T], f32, tag="pah", name=f"cx{w}")
                for sl in range(2):
                    s0 = sl * 512
                    nc.tensor.matmul(pj[:, s0:s0 + 512], wps[:, rt, :],
                                     hT[:, s0:s0 + 512], start=True, stop=True)
                    nc.tensor.matmul(cx[:, s0:s0 + 512], embs[:, rt, :],
                                     wfts[:, w, s0:s0 + 512], start=True, stop=True)
                inT = big.tile([128, P * T], f32, tag="inT", name=f"inT{w}")
                nc.scalar.activation(inT[0:64, :], pj, AF.Identity,
                                     bias=bps[:, rt, :], scale=1.0)
                nc.scalar.copy(inT[64:128, :], cx)
                # router softmax (no max subtraction; |logits| < 16)
                wsp = wsp_all[:, w * 16:(w + 1) * 16]
                wtiles = {}
                for u in range(8):
                    lg = ps.tile([128, 16], f32, tag="lg", name=f"lg{w}{u}")
                    nc.tensor.matmul(lg, inT[:, u * 128:(u + 1) * 128],
                                     wrs[:, rt, :], start=True, stop=True)
                    wun = big.tile([128, 16], f32, tag="wun", name=f"wun{w}{u}")
                    zr = big.tile([128, 1], f32, tag="zr", name=f"zr{w}{u}")
                    nc.scalar.activation(wun, lg, AF.Exp, bias=0.0, scale=1.0,
                                         accum_out=zr)
                    rz = big.tile([128, 1], f32, tag="rz", name=f"rz{w}{u}")
                    nc.vector.reciprocal(rz, zr)
                    wt = big.tile([128, 16], f32, tag="wt", bufs=8, name=f"wt{w}{u}")
                    nc.vector.tensor_scalar_mul(wt, wun, rz)
                    nc.tensor.matmul(wsp, ones128, wt, start=(u == 0), stop=(u == 7))
                    wtiles[u] = wt
                # t-product + transpose -> ttT [2][128, P*T]
                ttT = [big.tile([128, P * T], f32r, tag=f"ttT{g}", name=f"ttT{w}{g}")
                       for g in range(2)]
                for t in range(4):
                    for p in range(P):
                        u = p * 4 + t
                        tt = big.tile([128, 256], f32r, tag="tt", name=f"tt{w}{t}{p}")
                        wt = wtiles[u]
                        hh = hts[(t, p)]
                        nc.vector.tensor_mul(
                            tt.rearrange("q (n r) -> q n r", r=16),
                            _bc_last(wt, 16), _bc(hh, 16))
                        for g in range(2):
                            ptt = ps.tile([128, 128], f32r, tag="lg",
                                          name=f"ptt{w}{t}{p}{g}")
                            nc.tensor.transpose(
                                ptt, tt[:, g * 128:(g + 1) * 128], id32r)
                            nc.vector.tensor_copy(
                                ttT[g][:, p * T + t * 128: p * T + (t + 1) * 128], ptt)
                # restore
                rr = rq if w < 2 else rv
                if w < 2:  # Q^T / K^T  [d, row] (sum over p)
                    out = QTo if w == 0 else KTo
                    for c in range(8):
                        qp = ps.tile([128, T], f32, tag="pah", name=f"qp{w}{c}")
                        k = 0
                        for g in range(2):
                            for p in range(P):
                                nc.tensor.matmul(
                                    qp, rr[:, g, c * 128:(c + 1) * 128],
                                    ttT[g][:, p * T:(p + 1) * T],
                                    start=(k == 0), stop=(k == 3))
                                k += 1
                        qs = big.tile([128, T], f32, tag="qs", name=f"qs{w}{c}")
                        nc.scalar.copy(qs, qp)
                        nc.sync.dma_start(out=out[c * 128:(c + 1) * 128, :], in_=qs)
                else:  # V natural [row, d] (sum over p)
                    for t in range(4):
                        vp = ps.tile([128, D], f32, tag="pah", name=f"vp{t}")
                        for sl in range(2):
                            s0 = sl * 512
                            k = 0
                            for g in range(2):
                                for p in range(P):
                                    nc.tensor.matmul(
                                        vp[:, s0:s0 + 512],
                                        ttT[g][:, p * T + t * 128: p * T + (t + 1) * 128],
                                        rr[:, g, s0:s0 + 512],
                                        start=(k == 0), stop=(k == 3))
                                    k += 1
                        vs = big.tile([128, D], f32, tag="vs", name=f"vs{t}")
                        nc.scalar.copy(vs, vp)
                        nc.sync.dma_start(out=Vo[t * 128:(t + 1) * 128, :], in_=vs)
            # aux partial sums
            wsb = cst.tile([1, 48], f32, name="wsb")
            nc.vector.tensor_copy(wsb, wsp_all)
            nc.sync.dma_start(out=WSo, in_=wsb)
    nc.compile()
    return nc


# ---------------------------------------------------------------- launch B
def build_B():
    nc = bacc.Bacc("TRN2", target_bir_lowering=False, debug=False,
                   enable_asserts=False, num_devices=NC_)
    dt_ = nc.dram_tensor
    QTa = dt_("QTa", [4, 65, S], f32r, kind="ExternalInput").ap()
    KTa = dt_("KTa", [4, 65, S], f32r, kind="ExternalInput").ap()
    Va = dt_("Va", [4, S, 65], f32r, kind="ExternalInput").ap()
    WOs = dt_("WOs", [2, 128, D], f32r, kind="ExternalInput").ap()
    TRI = dt_("TRI", [128, 128], f32, kind="ExternalInput").ap()    # -1e30 strict upper
    TRI01 = dt_("TRI01", [128, 128], f32, kind="ExternalInput").ap()  # -1e30 strict lower (k>q)
    OUT = dt_("OUT", [S, D], f32, kind="ExternalOutput").ap()

    NQT = S // 128  # 16
    with tile.TileContext(nc) as tc:
        with tc.tile_pool(name="cst", bufs=1) as cst, \
             tc.tile_pool(name="pr", bufs=2) as pr, \
             tc.tile_pool(name="sm", bufs=4) as sm, \
             tc.tile_pool(name="ps", bufs=2, space="PSUM") as ps, \
             tc.tile_pool(name="psu", bufs=1, space="PSUM") as psu:

            tri = cst.tile([128, 128], f32, name="tri")
            nc.sync.dma_start(out=tri, in_=TRI)
            tri01 = cst.tile([128, 128], f32, name="tri01")
            nc.sync.dma_start(out=tri01, in_=TRI01)
            id32 = cst.tile([128, 128], f32, name="id32")
            make_identity(nc, id32)
            ones64f = cst.tile([1, 64], f32, name="ones64f")
            nc.vector.memset(ones64f, 1.0)
            ones64 = cst.tile([1, 64], f32r, name="ones64")
            nc.vector.tensor_copy(ones64, ones64f)
            wos = [cst.tile([128, D], f32r, name=f"wos{ch}") for ch in range(2)]
            for ch in range(2):
                nc.sync.dma_start(out=wos[ch], in_=WOs[ch])
            aot = [cst.tile([128, S], f32r, name=f"aot{half}") for half in range(2)]

            for pair in range(4):
                qta = pr.tile([65, S], f32r, tag="qta", name=f"qta{pair}")
                nc.sync.dma_start(out=qta, in_=QTa[pair])
                kta = pr.tile([65, S], f32r, tag="kta", name=f"kta{pair}")
                nc.sync.dma_start(out=kta, in_=KTa[pair])
                va = pr.tile([128, 16, 65], f32r, tag="va", name=f"va{pair}")
                nc.sync.dma_start(out=va,
                                  in_=Va[pair].rearrange("(j p) c -> p j c", p=128))

                # ---- pass 1: true row-max per q-tile (scores in [q,k] layout)
                for i in range(NQT):
                    nk = 128 * (i + 1)
                    m01 = sm.tile([128, 2], f32, tag="m01", name=f"m01{pair}{i}")
                    nhalves = 1 if i < 8 else 2
                    if nhalves == 1:
                        nc.vector.memset(m01[:, 1:2], -3.0e38)
                    for hh in range(nhalves):
                        k0 = 1024 * hh
                        kw = min(1024, nk - k0)
                        s1 = ps.tile([128, 1024], f32, tag="s1",
                                     name=f"s1{pair}{i}{hh}")
                        for sl in range((kw + 511) // 512):
                            w_ = min(512, kw - sl * 512)
                            nc.tensor.matmul(
                                s1[:, sl * 512: sl * 512 + w_],
                                qta[0:64, i * 128:(i + 1) * 128],
                                kta[0:64, k0 + sl * 512: k0 + sl * 512 + w_],
                                start=True, stop=True)
                        if i // 8 == hh:  # diag block inside this half
                            off = 128 * i - k0
                            nc.vector.tensor_add(
                                s1[:, off:off + 128], s1[:, off:off + 128], tri)
                        nc.vector.reduce_max(out=m01[:, hh:hh + 1],
                                             in_=s1[:, 0:kw], axis=AX.X)
                    m = sm.tile([128, 1], f32, tag="m", name=f"m{pair}{i}")
                    nc.vector.reduce_max(out=m, in_=m01, axis=AX.X)
                    mt = ps.tile([1, 128], f32, tag="s1", name=f"mt{pair}{i}")
                    nc.tensor.transpose(mt, m, id32)
                    nc.vector.tensor_scalar_mul(
                        qta[64:65, i * 128:(i + 1) * 128], mt, -1.0)

                # ---- pass 2 + AV (scores transposed, max folded via aug row)
                U = psu.tile([65, S], f32, tag="U", name=f"U{pair}")
                for j in range(NQT):
                    q0 = 128 * j
                    first = True
                    qq = q0
                    while qq < S:
                        w_ = min(512, S - qq)
                        st = ps.tile([128, 512], f32, tag="s1",
                                     name=f"st{pair}{j}{qq}")
                        nc.tensor.matmul(st[:, :w_],
                                         kta[:, q0:q0 + 128],
                                         qta[:, qq:qq + w_],
                                         start=True, stop=True)
                        if first:
                            nc.vector.tensor_add(st[:, 0:128], st[:, 0:128], tri01)
                        et = sm.tile([128, 512], f32r, tag="et",
                                     name=f"et{pair}{j}{qq}")
                        nc.scalar.activation(et[:, :w_], st[:, :w_], AF.Exp,
                                             bias=0.0, scale=0.125)
                        nc.tensor.matmul(U[:, qq:qq + w_],
                                         va[:, j, :], et[:, :w_],
                                         start=(j == 0), stop=(j == NQT - 1))
                        first = False
                        qq += w_

                # ---- normalize -> aot
                half = pair // 2
                r0 = 64 * (pair % 2)
                for sl in range(4):
                    s0 = sl * 512
                    rz = sm.tile([1, 512], f32r, tag="rz", name=f"rz{pair}{sl}")
                    with nc.allow_low_precision(reason="f32r is f32 bits"):
                        nc.vector.reciprocal(rz, U[64:65, s0:s0 + 512])
                    bc = ps.tile([64, 512], f32, tag="s1", name=f"bc{pair}{sl}")
                    nc.tensor.matmul(bc, ones64, rz, start=True, stop=True)
                    bcs = sm.tile([64, 512], f32, tag="bcs", name=f"bcs{pair}{sl}")
                    nc.vector.tensor_copy(bcs, bc)
                    nc.vector.tensor_mul(
                        aot[half][r0:r0 + 64, s0:s0 + 512],
                        U[0:64, s0:s0 + 512], bcs)

            # ---- W_O partial
            for i in range(NQT):
                wo = ps.tile([128, D], f32, tag="s1", name=f"wo{i}")
                for sl in range(2):
                    s0 = sl * 512
                    for ch in range(2):
                        nc.tensor.matmul(wo[:, s0:s0 + 512],
                                         aot[ch][:, i * 128:(i + 1) * 128],
                                         wos[ch][:, s0:s0 + 512],
                                         start=(ch == 0), stop=(ch == 1))
                wsb = sm.tile([128, D], f32, tag="wsb", name=f"wsb{i}")
                nc.scalar.copy(wsb, wo)
                nc.sync.dma_start(out=OUT[i * 128:(i + 1) * 128, :], in_=wsb)
    nc.compile()
    return nc


_NC_A, _NC_B = None, None
_EXEC_CACHE = {}


def _run_cached(key, nc, in_maps):
    """run_bass_via_pjrt with the jitted executable cached across calls."""
    import jax
    from jax.sharding import Mesh, PartitionSpec
    from jax.experimental.shard_map import shard_map
    from concourse import bass2jax, mybir as mb

    if key not in _EXEC_CACHE:
        bass2jax.install_neuronx_cc_hook()
        pname = nc.partition_id_tensor.name if nc.partition_id_tensor else None
        in_names, out_names, out_avals = [], [], []
        for alloc in nc.m.functions[0].allocations:
            if not isinstance(alloc, mb.MemoryLocationSet):
                continue
            name = alloc.memorylocations[0].name
            if alloc.kind == "ExternalInput":
                if name != pname:
                    in_names.append(name)
            elif alloc.kind == "ExternalOutput":
                out_names.append(name)
                out_avals.append(jax.core.ShapedArray(
                    tuple(alloc.tensor_shape), mb.dt.np(alloc.dtype)))
        n_params = len(in_names)
        all_names = in_names + out_names
        if pname is not None:
            all_names = all_names + [pname]

        def _body(*args):
            operands = list(args)
            if pname is not None:
                operands.append(bass2jax.partition_id_tensor())
            outs = bass2jax._bass_exec_p.bind(
                *operands, out_avals=tuple(out_avals), in_names=tuple(all_names),
                out_names=tuple(out_names), lowering_input_output_aliases=(),
                sim_require_finite=True, sim_require_nnan=True, nc=nc)
            return tuple(outs)

        devices = jax.devices()[:NC_]
        mesh = Mesh(np.asarray(devices), ("core",))
        n_outs = len(out_names)
        sharded = jax.jit(
            shard_map(_body, mesh=mesh,
                      in_specs=(PartitionSpec("core"),) * (n_params + n_outs),
                      out_specs=(PartitionSpec("core"),) * n_outs,
                      check_rep=False),
            donate_argnums=tuple(range(n_params, n_params + n_outs)),
            keep_unused=True)
        _EXEC_CACHE[key] = (sharded, in_names, out_names, out_avals, n_params)

    sharded, in_names, out_names, out_avals, n_params = _EXEC_CACHE[key]
    concat_in = [np.concatenate([np.asarray(in_maps[c][nm]) for c in range(NC_)],
                                axis=0) for nm in in_names]
    concat_zeros = [np.zeros((NC_ * av.shape[0], *av.shape[1:]), av.dtype)
                    for av in out_avals]
    out_arrs = sharded(*concat_in, *concat_zeros)
    return [
        {nm: np.asarray(out_arrs[i]).reshape(NC_, *out_avals[i].shape)[c]
         for i, nm in enumerate(out_names)}
        for c in range(NC_)
    ]


def kernel(x, fqk_wQ, fqk_wK, fv_w, f_neurons, r_neurons, fqk_emb, fv_emb,
           Wp_qk, bp_qk, Wp_v, bp_v, Wr_qk, Wr_v, W_O):
    global _NC_A, _NC_B
    if _NC_A is None:
        _NC_A = build_A()
        _NC_B = build_B()

    f32n = np.float32
    x = np.asarray(x, f32n)
    xf = x.reshape(B * S, D)
    Fa = np.ascontiguousarray(
        np.asarray(f_neurons, f32n).transpose(1, 0, 2).reshape(D, 1024))
    rn = np.asarray(r_neurons, f32n)
    Rqk = np.ascontiguousarray(rn[:NRQK].reshape(256, D))
    Rv = np.ascontiguousarray(rn[NRQK:].reshape(256, D))
    wf_all = np.stack([np.asarray(fqk_wQ, f32n), np.asarray(fqk_wK, f32n),
                       np.asarray(fv_w, f32n)])          # [3,P,B,S,32]
    wf_flat = wf_all.reshape(3, P, B * S, 32)
    emb = np.stack([np.asarray(fqk_emb, f32n), np.asarray(fv_emb, f32n)])
    Wp = np.stack([np.asarray(Wp_qk, f32n), np.asarray(Wp_v, f32n)])
    Wr = np.stack([np.asarray(Wr_qk, f32n), np.asarray(Wr_v, f32n)])
    cb = np.stack([np.asarray(bp_qk, f32n).reshape(DS, 1),
                   np.asarray(bp_v, f32n).reshape(DS, 1)])

    in_maps_a = []
    for c in range(NC_):
        rows = slice(c * T, (c + 1) * T)
        wfc = np.ascontiguousarray(wf_flat[:, :, rows, :])           # [3,P,T,32]
        wfTc = np.ascontiguousarray(
            wfc.transpose(0, 3, 1, 2).reshape(3, 32, P * T))
        in_maps_a.append(dict(
            xT=np.ascontiguousarray(xf[rows].T), Fa=Fa, Rqk=Rqk, Rv=Rv,
            wf=wfc, wfT=wfTc, emb=emb, Wp=Wp, Wr=Wr, bp=cb))
    resA_list = _run_cached("A", _NC_A, in_maps_a)

    class _R:
        pass
    resA = _R(); resA.results = resA_list

    QT = np.empty((B, D, S), f32n)
    KT = np.empty((B, D, S), f32n)
    V = np.empty((B, S, D), f32n)
    ws = np.zeros((3, 16), f32n)
    for c in range(NC_):
        b, cc = c // 4, c % 4
        QT[b][:, cc * T:(cc + 1) * T] = resA.results[c]["QTo"]
        KT[b][:, cc * T:(cc + 1) * T] = resA.results[c]["KTo"]
        V[b][cc * T:(cc + 1) * T, :] = resA.results[c]["Vo"]
        ws += resA.results[c]["WSo"].reshape(3, 16)
    mean_p = ws / f32n(P * B * S)
    aux = f32n(16.0) * (mean_p * mean_p).sum(axis=1)
    restore_aux_loss = np.float32(aux.sum())

    # q_norm scale (reference: 1 unless ||Q|| <= 1e-6)
    qn = np.sqrt((QT ** 2).sum(axis=1))                      # [B, S]
    scale = np.where(qn > 1e-6, f32n(1.0), qn * f32n(1e-6))

    TRIm = np.where(np.arange(128)[:, None] < np.arange(128)[None, :],
                    f32n(NEG), f32n(0.0)).astype(f32n)
    TRI01 = np.where(np.arange(128)[:, None] > np.arange(128)[None, :],
                     f32n(NEG), f32n(0.0)).astype(f32n)
    WOn = np.asarray(W_O, f32n)

    in_maps_b = []
    for c in range(NC_):
        b, hb = c // 4, c % 4
        qa = np.zeros((4, 65, S), f32n)
        ka = np.zeros((4, 65, S), f32n)
        va = np.zeros((4, S, 65), f32n)
        for t in range(4):
            h = 4 * hb + t
            qa[t, 0:64] = QT[b][h * DH:(h + 1) * DH]
            ka[t, 0:64] = KT[b][h * DH:(h + 1) * DH]
            ka[t, 64] = 1.0
            va[t, :, 0:64] = V[b][:, h * DH:(h + 1) * DH]
            va[t, :, 64] = 1.0
        wos = np.ascontiguousarray(
            WOn[hb * 256:(hb + 1) * 256].reshape(2, 128, D))
        in_maps_b.append(dict(QTa=qa, KTa=ka, Va=va, WOs=wos,
                              TRI=TRIm, TRI01=TRI01))
    resB = _R(); resB.results = _run_cached("B", _NC_B, in_maps_b)

    out = np.zeros((B, S, D), f32n)
    for c in range(NC_):
        out[c // 4] += resB.results[c]["OUT"]
    if not np.all(scale == 1.0):
        out *= scale[:, :, None]
    return out, restore_aux_loss
